# revision 2
# baseline (speedup 1.0000x reference)
"""Multi-head causal self-attention (B=2, T=2048, D=1024, H=16, Dh=64) on 8
Trainium2 NeuronCores.

Sharding (Megatron-style tensor parallel over heads):
  - Each core owns 2 heads (core c -> heads 2c, 2c+1) for both batch rows.
  - w_qkv column-sharded: each core gets its heads' q/k/v columns
    ([1024, 128] each, cast to bf16 on host).
  - w_proj row-sharded: each core gets the rows matching its heads
    ([128, 1024] bf16); cores emit partial projection outputs which the
    host sums (plus the bias terms, folded exactly on the host).
  - x is replicated to all cores (passed pre-transposed as xT [1024, 4096]
    fp32 so the contraction dim lands on SBUF partitions; cast to bf16
    on-device).

Device-side per core:
  xT -> (cast bf16) -> qT/kT/vT = W^T x^T via PE (fp32 PSUM accum)
  vT -> PE-transpose -> V tiles with interleaved ones-columns
  per (batch, head, 128-query block): S^T = K Q^T (causal block-skipped),
  P^T = exp(S^T/8) via one wide ScalarE pass per PSUM group, diagonal
  block masked with a triangular multiply, out = P V with an appended
  ones column giving the softmax denominator for free, normalize, PE
  transpose into the projection layout, then y_partial = attn_out W_proj.

Softmax max-subtraction is omitted deliberately: scores are bounded
(|s| < ~4 for this problem's 0.02-scaled weights), so exp is safe in fp32
and the result is mathematically identical to jax.nn.softmax.
"""

import numpy as np
import ml_dtypes

import concourse.bacc as bacc
import concourse.bass as bass
import concourse.mybir as mybir
import concourse.tile as tile
from concourse.bass_utils import run_bass_kernel_spmd
from concourse.masks import make_identity

N_CORES = 8
B = 2
T = 2048
D = 1024
H = 16
DH = 64
TA = B * T  # 4096 rows total
P = 128
NQB = T // P  # 16 query blocks per batch
KC = D // P  # 8 contraction chunks for qkv
BF = mybir.dt.bfloat16
F32 = mybir.dt.float32

_CACHED_NC = None


def build_nc():
    """Build the per-core Bass program (identical on all 8 cores)."""
    nc = bacc.Bacc("TRN2", target_bir_lowering=False, debug=False, num_devices=N_CORES)

    xT_in = nc.dram_tensor("xT", [D, TA], F32, kind="ExternalInput").ap()
    wq_in = nc.dram_tensor("wq", [D, P], BF, kind="ExternalInput").ap()
    wk_in = nc.dram_tensor("wk", [D, P], BF, kind="ExternalInput").ap()
    wv_in = nc.dram_tensor("wv", [D, P], BF, kind="ExternalInput").ap()
    bq_in = nc.dram_tensor("bq", [P, 1], F32, kind="ExternalInput").ap()
    bk_in = nc.dram_tensor("bk", [P, 1], F32, kind="ExternalInput").ap()
    wp_in = nc.dram_tensor("wp", [P, D], BF, kind="ExternalInput").ap()
    mask_in = nc.dram_tensor("mask", [P, P], BF, kind="ExternalInput").ap()
    y_out = nc.dram_tensor("y", [TA, D], BF, kind="ExternalOutput").ap()

    with tile.TileContext(nc) as tc:
        with (
            tc.tile_pool(name="const", bufs=1) as const,
            tc.tile_pool(name="xstage", bufs=4) as xstage,
            tc.tile_pool(name="xts", bufs=1) as xts,
            tc.tile_pool(name="qkv", bufs=1) as qkv,
            tc.tile_pool(name="ptp", bufs=4) as ptp,
            tc.tile_pool(name="osml", bufs=4) as osml,
            tc.tile_pool(name="rcp", bufs=4) as rcp,
            tc.tile_pool(name="ystage", bufs=3) as ystage,
            tc.tile_pool(name="ps_mm", bufs=2, space="PSUM") as ps_mm,
            tc.tile_pool(name="ps_st", bufs=2, space="PSUM") as ps_st,
            tc.tile_pool(name="ps_sm", bufs=2, space="PSUM") as ps_sm,
        ):
            # ---- constants ----
            ident = const.tile([P, P], BF)
            make_identity(nc, ident[:])
            mask_sb = const.tile([P, P], BF)
            nc.sync.dma_start(mask_sb[:], mask_in[:])
            bq_sb = const.tile([P, 1], F32)
            nc.sync.dma_start(bq_sb[:], bq_in[:])
            bk_sb = const.tile([P, 1], F32)
            nc.sync.dma_start(bk_sb[:], bk_in[:])
            wp_sb = const.tile([P, D], BF)
            nc.sync.dma_start(wp_sb[:], wp_in[:])
            # qkv weight chunks as matmul lhsT tiles [K=128 D-rows, 128 feats]
            w_sb = {}
            for name, ap in (("q", wq_in), ("k", wk_in), ("v", wv_in)):
                w = const.tile([P, KC, P], BF, name=f"w{name}sb")
                for c in range(KC):
                    nc.sync.dma_start(w[:, c, :], ap[c * P : (c + 1) * P, :])
                w_sb[name] = w

            # ---- xT load + cast to bf16 ----
            # xT_sb[:, c, :] holds rows [128c, 128c+128) of x^T, bf16.
            xT_sb = xts.tile([P, KC, TA], BF)
            NSPLIT = 4
            SW = TA // NSPLIT  # 1024 cols per split
            for s in range(NSPLIT):
                for c in range(KC):
                    xs = xstage.tile([P, SW], F32)
                    nc.sync.dma_start(
                        xs[:], xT_in[c * P : (c + 1) * P, s * SW : (s + 1) * SW]
                    )
                    nc.vector.tensor_copy(xT_sb[:, c, s * SW : (s + 1) * SW], xs[:])

            # ---- qkv projections: qT/kT/vT [128 feats, b, 2048] ----
            qT_sb = qkv.tile([P, B, T], BF)
            kT_sb = qkv.tile([P, B, T], BF)
            vT_sb = qkv.tile([P, B, T], BF)
            NTC = TA // 512  # 8 T-chunks of 512
            for blk, wt, dst, bias in (
                ("q", w_sb["q"], qT_sb, bq_sb),
                ("k", w_sb["k"], kT_sb, bk_sb),
                ("v", w_sb["v"], vT_sb, None),
            ):
                for tp2 in range(NTC // 2):  # pairs of T-chunks share ldweights
                    pst = [
                        ps_mm.tile([P, 512], F32, name=f"ps{blk}", tag="psq")
                        for _ in range(2)
                    ]
                    for c in range(KC):
                        for i in range(2):
                            tcg = tp2 * 2 + i
                            nc.tensor.matmul(
                                pst[i][:],
                                w_sb[blk][:, c, :],
                                xT_sb[:, c, tcg * 512 : (tcg + 1) * 512],
                                start=(c == 0),
                                stop=(c == KC - 1),
                            )
                    for i in range(2):
                        tcg = tp2 * 2 + i
                        b = tcg // (NTC // B)
                        col = (tcg % (NTC // B)) * 512
                        d = dst[:, b, col : col + 512]
                        if bias is not None:
                            nc.vector.tensor_scalar(
                                d, pst[i][:], bias[:], None, op0=mybir.AluOpType.add
                            )
                        else:
                            nc.vector.tensor_copy(d, pst[i][:])

            # ---- V fixup: vT -> V2 tiles [1|Vh0|1|Vh1] per (b, k-chunk) ----
            V2 = qkv.tile([P, B, NQB, 130], BF)
            nc.vector.memset(V2[:, :, :, 0], 1.0)
            nc.vector.memset(V2[:, :, :, 65], 1.0)
            for b in range(B):
                for s in range(NQB):
                    tpp = ps_sm.tile([P, P], BF, name="tpv", tag="sm")
                    nc.tensor.transpose(
                        tpp[:], vT_sb[:, b, s * P : (s + 1) * P], ident[:]
                    )
                    nc.vector.tensor_copy(V2[:, b, s, 1:65], tpp[:, 0:DH])
                    nc.vector.tensor_copy(V2[:, b, s, 66:130], tpp[:, DH:P])

            # ---- attention ----
            attn_oT = qkv.tile([P, TA], BF)
            for b in range(B):
                for qi in range(NQB):
                    nk = qi + 1  # causal: only key chunks 0..qi
                    pt = {}
                    for h in (0, 1):
                        pt[h] = ptp.tile([P, T], BF, name="ptt", tag="pt")
                    for g in range(0, nk, 8):  # PSUM groups of <=8 chunks
                        gn = min(8, nk - g)
                        st = {}
                        for h in (0, 1):
                            st[h] = ps_st.tile([P, 1024], F32, name="st", tag="st")
                        for j in range(gn):
                            c = g + j
                            for h in (0, 1):
                                hp = h * DH
                                nc.tensor.matmul(
                                    st[h][:, j * P : (j + 1) * P],
                                    kT_sb[hp : hp + DH, b, c * P : (c + 1) * P],
                                    qT_sb[hp : hp + DH, b, qi * P : (qi + 1) * P],
                                    start=(j % 4 == 0),
                                    stop=(j % 4 == 3 or j == gn - 1),
                                )
                        for h in (0, 1):
                            nc.scalar.activation(
                                pt[h][:, g * P : (g + gn) * P],
                                st[h][:, 0 : gn * P],
                                mybir.ActivationFunctionType.Exp,
                                scale=0.125,
                            )
                    for h in (0, 1):
                        # mask the diagonal block (upper-tri keep in [k, q])
                        nc.vector.tensor_mul(
                            pt[h][:, qi * P : (qi + 1) * P],
                            pt[h][:, qi * P : (qi + 1) * P],
                            mask_sb[:],
                        )
                        pv = ps_sm.tile([P, 65], F32, name="pv", tag="sm")
                        for c in range(nk):
                            nc.tensor.matmul(
                                pv[:],
                                pt[h][:, c * P : (c + 1) * P],
                                V2[:, b, c, h * 65 : h * 65 + 65],
                                start=(c == 0),
                                stop=(c == nk - 1),
                            )
                        r = rcp.tile([P, 1], F32)
                        nc.vector.reciprocal(r[:], pv[:, 0:1])
                        osb = osml.tile([P, DH], BF)
                        nc.vector.tensor_scalar_mul(osb[:], pv[:, 1:65], r[:])
                        top = ps_sm.tile([P, P], BF, name="top", tag="sm")
                        hp = h * DH
                        nc.tensor.transpose(
                            top[hp : hp + DH, :],
                            osb[:],
                            ident[:],
                            tile_position=(0, hp),
                        )
                        nc.vector.tensor_copy(
                            attn_oT[hp : hp + DH, b * T + qi * P : b * T + (qi + 1) * P],
                            top[hp : hp + DH, :],
                        )

            # ---- projection: y_partial[t, :] = attn_out @ w_proj_slice ----
            for tt in range(TA // P):
                ys = ystage.tile([P, D], BF)
                for nh in range(2):
                    psp = ps_mm.tile([P, 512], F32, name="psp", tag="psq")
                    nc.tensor.matmul(
                        psp[:],
                        attn_oT[:, tt * P : (tt + 1) * P],
                        wp_sb[:, nh * 512 : (nh + 1) * 512],
                        start=True,
                        stop=True,
                    )
                    if nh == 0:
                        nc.vector.tensor_copy(ys[:, nh * 512 : (nh + 1) * 512], psp[:])
                    else:
                        nc.scalar.copy(ys[:, nh * 512 : (nh + 1) * 512], psp[:])
                nc.sync.dma_start(y_out[tt * P : (tt + 1) * P, :], ys[:])

    nc.compile()
    return nc


def get_nc():
    global _CACHED_NC
    if _CACHED_NC is None:
        _CACHED_NC = build_nc()
    return _CACHED_NC


def make_in_maps(x, w_qkv, b_qkv, w_proj):
    x = np.asarray(x, dtype=np.float32).reshape(TA, D)
    w_qkv = np.asarray(w_qkv, dtype=np.float32)
    b_qkv = np.asarray(b_qkv, dtype=np.float32)
    w_proj = np.asarray(w_proj, dtype=np.float32)
    xT = np.ascontiguousarray(x.T)  # [D, TA] fp32, replicated
    mask = np.triu(np.ones((P, P))).astype(ml_dtypes.bfloat16)  # keep k<=q in [k,q]
    bf = ml_dtypes.bfloat16
    in_maps = []
    for c in range(N_CORES):
        lo = 2 * c * DH  # first feature column of this core's 2 heads
        in_maps.append(
            {
                "xT": xT,
                "wq": np.ascontiguousarray(w_qkv[:, lo : lo + P]).astype(bf),
                "wk": np.ascontiguousarray(w_qkv[:, D + lo : D + lo + P]).astype(bf),
                "wv": np.ascontiguousarray(w_qkv[:, 2 * D + lo : 2 * D + lo + P]).astype(
                    bf
                ),
                "bq": np.ascontiguousarray(b_qkv[lo : lo + P][:, None]),
                "bk": np.ascontiguousarray(b_qkv[D + lo : D + lo + P][:, None]),
                "wp": np.ascontiguousarray(w_proj[lo : lo + P, :]).astype(bf),
                "mask": mask,
            }
        )
    return in_maps


def gather(results, b_qkv, w_proj, b_proj):
    b_qkv = np.asarray(b_qkv, dtype=np.float32)
    w_proj = np.asarray(w_proj, dtype=np.float32)
    b_proj = np.asarray(b_proj, dtype=np.float32)
    y = np.zeros((TA, D), dtype=np.float32)
    for c in range(N_CORES):
        y += np.asarray(results[c]["y"], dtype=np.float32)
    # exact host-side fold of the v-bias and projection bias:
    # softmax rows sum to 1, so the v-bias passes through attention intact.
    y += b_qkv[2 * D : 3 * D] @ w_proj + b_proj
    return y.reshape(B, T, D)


def run(x, w_qkv, b_qkv, w_proj, b_proj, trace=False, **spmd_kwargs):
    nc = get_nc()
    in_maps = make_in_maps(x, w_qkv, b_qkv, w_proj)
    res = run_bass_kernel_spmd(
        nc, in_maps, list(range(N_CORES)), trace=trace, **spmd_kwargs
    )
    return gather(res.results, b_qkv, w_proj, b_proj), res


def kernel(x, w_qkv, b_qkv, w_proj, b_proj):
    y, _ = run(x, w_qkv, b_qkv, w_proj, b_proj)
    return y


# revision 39
# speedup vs baseline: 11.6130x; 11.6130x over previous
"""Multi-head causal self-attention (B=2, T=2048, D=1024, H=16, Dh=64) on 8
Trainium2 NeuronCores.

Sharding (Megatron-style tensor parallel over heads):
  - Each core owns 2 heads (core c -> heads 2c, 2c+1) for both batch rows.
  - w_qkv column-sharded: each core gets its heads' q/k/v columns
    ([1024, 128] each, cast to bf16 on host).
  - w_proj row-sharded: each core gets the rows matching its heads
    ([128, 1024] bf16); cores emit partial projection outputs which the
    host sums (plus the bias terms, folded exactly on the host).
  - x is replicated to all cores (passed pre-transposed as xT [1024, 4096]
    fp32 so the contraction dim lands on SBUF partitions; cast to bf16
    on-device).

Device-side per core:
  xT -> (cast bf16 on GpSimd) -> qT/kT/vT = W^T x^T via PE (fp32 PSUM)
  vT -> PE-transpose -> V tiles with a prepended ones-column per head
  per (batch, 256-query superblock): S^T = K Q^T in [keys, queries]
  layout with N=256 moving tiles (causally block-skipped; the two
  diagonal chunks get the causal mask folded in as an extra
  identity.T @ (-1600 mask) matmul accumulate), P^T = exp(S^T/8) via one
  wide ScalarE pass per PSUM group, out = P V per 128-query sub-block
  with the ones column yielding the softmax denominator in column 0,
  normalize on DVE, PE-transpose into the projection layout
  (tile_position places head 1 at partitions 64..127), then
  y_partial = attn_out @ w_proj_slice.

The emission order software-pipelines blocks at distance 2 (scores of
block i before PV/normalize/projection of block i-2) and stage-majors
the per-superblock output work so the in-order PE never blocks on DVE.

Softmax max-subtraction is omitted deliberately: scores are bounded
(|s| < ~4 for this problem's 0.02-scaled weights), so exp is safe in fp32
and the result is mathematically identical to jax.nn.softmax.
"""

import numpy as np
import ml_dtypes

import concourse.bacc as bacc
import concourse.bass as bass
import concourse.mybir as mybir
import concourse.tile as tile
from concourse.bass_utils import run_bass_kernel_spmd
from concourse.masks import make_identity

N_CORES = 8
B = 2
T = 2048
D = 1024
H = 16
DH = 64
TA = B * T  # 4096 rows total
P = 128
NQB = T // P  # 16 query blocks per batch
KC = D // P  # 8 contraction chunks for qkv
BF = mybir.dt.bfloat16
F32 = mybir.dt.float32

_CACHED_NC = None
DEBUG_DUMP = False


def build_nc():
    """Build the per-core Bass program (identical on all 8 cores)."""
    nc = bacc.Bacc("TRN2", target_bir_lowering=False, debug=False, num_devices=N_CORES)

    xT_in = nc.dram_tensor("xT", [D, TA], F32, kind="ExternalInput").ap()
    wq_in = nc.dram_tensor("wq", [D, P], BF, kind="ExternalInput").ap()
    wk_in = nc.dram_tensor("wk", [D, P], BF, kind="ExternalInput").ap()
    wv_in = nc.dram_tensor("wv", [D, P], BF, kind="ExternalInput").ap()
    bq_in = nc.dram_tensor("bq", [P, 1], F32, kind="ExternalInput").ap()
    bk_in = nc.dram_tensor("bk", [P, 1], F32, kind="ExternalInput").ap()
    wp_in = nc.dram_tensor("wp", [P, D], BF, kind="ExternalInput").ap()
    mask_in = nc.dram_tensor("mask", [P, 2 * 256], BF, kind="ExternalInput").ap()
    y_out = nc.dram_tensor("y", [TA, D], BF, kind="ExternalOutput").ap()
    dbg_out = None
    if DEBUG_DUMP:
        dbg_out = nc.dram_tensor("dbg", [P, TA], BF, kind="ExternalOutput").ap()
        dbg_pt = nc.dram_tensor("dbg_pt", [P, 16 * 256], BF, kind="ExternalOutput").ap()

    with tile.TileContext(nc) as tc:
        with (
            tc.tile_pool(name="const", bufs=1) as const,
            tc.tile_pool(name="xstage", bufs=10) as xstage,
            tc.tile_pool(name="xts", bufs=1) as xts,
            tc.tile_pool(name="qkv", bufs=1) as qkv,
            tc.tile_pool(name="ptp", bufs=6) as ptp,
            tc.tile_pool(name="osml", bufs=8) as osml,
            tc.tile_pool(name="rcp", bufs=8) as rcp,
            tc.tile_pool(name="ystage", bufs=3) as ystage,
            tc.tile_pool(name="ps_mm", bufs=2, space="PSUM") as ps_mm,
            tc.tile_pool(name="ps_st", bufs=2, space="PSUM") as ps_st,
            tc.tile_pool(name="ps_sm", bufs=2, space="PSUM") as ps_sm,
        ):
            # ---- constants (wp/mask DMAs deferred below x split 0) ----
            ident = const.tile([P, P], BF)
            make_identity(nc, ident[:])
            bq_sb = const.tile([P, 1], F32)
            nc.sync.dma_start(bq_sb[:], bq_in[:])
            bk_sb = const.tile([P, 1], F32)
            nc.sync.dma_start(bk_sb[:], bk_in[:])
            # qkv weight chunks as matmul lhsT tiles [K=128 D-rows, 128 feats]
            w_sb = {}
            for name, ap in (("q", wq_in), ("k", wk_in), ("v", wv_in)):
                w = const.tile([P, KC, P], BF, name=f"w{name}sb")
                for c in range(KC):
                    nc.sync.dma_start(w[:, c, :], ap[c * P : (c + 1) * P, :])
                w_sb[name] = w

            # ---- xT load + cast to bf16 (gpsimd does the cast) ----
            # xT_sb[:, c, :] holds rows [128c, 128c+128) of x^T, bf16.
            xT_sb = xts.tile([P, KC, TA], BF)
            NSPLIT = 8
            SW = TA // NSPLIT  # 512 cols per split
            mask_sb = const.tile([P, 2 * 256], BF)
            wp_sb = const.tile([P, D], BF)
            for s in range(NSPLIT):
                for c in range(KC):
                    xs = xstage.tile([P, SW], F32)
                    dma_eng = nc.scalar if s == 0 else nc.sync
                    dma_eng.dma_start(
                        xs[:], xT_in[c * P : (c + 1) * P, s * SW : (s + 1) * SW]
                    )
                    eng = nc.vector if (s == 0 and c % 2 == 0) else nc.gpsimd
                    eng.tensor_copy(xT_sb[:, c, s * SW : (s + 1) * SW], xs[:])
                if s == 0:  # needed later than qkv; keep off the startup queue
                    nc.sync.dma_start(mask_sb[:], mask_in[:])
                    nc.sync.dma_start(wp_sb[:], wp_in[:])

            # ---- qkv projections: qT/kT/vT [128 feats, b, 2048] ----
            # T-chunk-major so batch 0 completes before batch 1 starts and
            # attention(b0) can overlap qkv(b1).
            qT_sb = qkv.tile([P, B, T], BF)
            kT_sb = qkv.tile([P, B, T], BF)
            vT_sb = qkv.tile([P, B, T], BF)
            # V2 per (b, key-chunk): [1 | V_h0 (64) | 1 | V_h1 (64)]
            V2 = qkv.tile([P, B, NQB, 130], BF)
            nc.vector.memset(V2[:, :, :, 0], 1.0)
            nc.vector.memset(V2[:, :, :, 65], 1.0)
            attn_oT = qkv.tile([P, TA], BF)
            NTC = TA // 512  # 8 T-chunks of 512
            SQ = 256  # superblock query count
            NSB = T // SQ  # 8 superblocks per batch

            def qkv_tchunk(tcg):
                b = tcg // (NTC // B)
                col = (tcg % (NTC // B)) * 512
                for blk, dst, bias in (
                    ("q", qT_sb, bq_sb),
                    ("k", kT_sb, bk_sb),
                    ("v", vT_sb, None),
                ):
                    pst = ps_mm.tile([P, 512], F32, name="psqkv", tag="psq")
                    for c in range(KC):
                        nc.tensor.matmul(
                            pst[:],
                            w_sb[blk][:, c, :],
                            xT_sb[:, c, tcg * 512 : (tcg + 1) * 512],
                            start=(c == 0),
                            stop=(c == KC - 1),
                        )
                    d = dst[:, b, col : col + 512]
                    if bias is not None:
                        nc.vector.tensor_scalar(
                            d, pst[:], bias[:], None, op0=mybir.AluOpType.add
                        )
                    else:
                        nc.vector.tensor_copy(d, pst[:])
                # V fixup for the 4 key chunks this T-chunk covers
                bs = (tcg % (NTC // B)) * 4
                for s in range(bs, bs + 4):
                    tpp = ps_mm.tile([P, P], BF, name="tpv", tag="psq")
                    nc.tensor.transpose(
                        tpp[:], vT_sb[:, b, s * P : (s + 1) * P], ident[:]
                    )
                    nc.vector.tensor_copy(V2[:, b, s, 1:65], tpp[:, 0:DH])
                    nc.vector.tensor_copy(V2[:, b, s, 66:130], tpp[:, DH:P])

            def proj_tchunk(tt):
                # y_partial rows [128*tt, 128*tt+128) = attn_out @ w_proj_slice
                ys = ystage.tile([P, D], BF)
                for nh in range(2):
                    psp = ps_mm.tile([P, 512], F32, name="psp", tag="psq")
                    nc.tensor.matmul(
                        psp[:],
                        attn_oT[:, tt * P : (tt + 1) * P],
                        wp_sb[:, nh * 512 : (nh + 1) * 512],
                        start=True,
                        stop=True,
                    )
                    if nh == 0:
                        nc.vector.tensor_copy(ys[:, nh * 512 : (nh + 1) * 512], psp[:])
                    else:
                        nc.scalar.copy(ys[:, nh * 512 : (nh + 1) * 512], psp[:])
                nc.sync.dma_start(y_out[tt * P : (tt + 1) * P, :], ys[:])

            def attn_scores(b, sq):
                """S^T matmuls + exp for one 256-query superblock: PE -> ACT.

                S^T chunks are [128 keys, 256 queries]; the causal mask for
                the two diagonal chunks is folded in as an extra
                identity.T @ mneg matmul accumulate, so exp() zeroes the
                masked entries with no separate masking pass.
                """
                nk = 2 * sq + 2  # causal: key chunks 0..2*sq+1
                pt = {}
                for h in (0, 1):
                    pt[h] = ptp.tile([P, NQB, SQ], BF, name="ptt", tag="pt")
                for g in range(0, nk, 4):  # PSUM groups of <=4 chunks
                    gn = min(4, nk - g)
                    st = {}
                    for h in (0, 1):
                        st[h] = ps_st.tile([P, 1024], F32, name="st", tag="st")
                    for j in range(gn):
                        c = g + j
                        diag = c >= nk - 2  # last two chunks touch the diagonal
                        for h in (0, 1):
                            hp = h * DH
                            nc.tensor.matmul(
                                st[h][:, j * SQ : (j + 1) * SQ],
                                kT_sb[hp : hp + DH, b, c * P : (c + 1) * P],
                                qT_sb[hp : hp + DH, b, sq * SQ : (sq + 1) * SQ],
                                start=(j % 2 == 0),
                                stop=(j % 2 == 1 or j == gn - 1) and not diag,
                            )
                            if diag:
                                m = (c - (nk - 2)) * SQ
                                nc.tensor.matmul(
                                    st[h][:, j * SQ : (j + 1) * SQ],
                                    ident[:],
                                    mask_sb[:, m : m + SQ],
                                    start=False,
                                    stop=(c == nk - 1),
                                )
                    for h in (0, 1):
                        nc.scalar.activation(
                            pt[h][:, g : g + gn, :],
                            st[h][:, 0 : gn * SQ],
                            mybir.ActivationFunctionType.Exp,
                            scale=0.125,
                        )
                return pt

            dbg_holder = {}

            def attn_output(b, sq, pt):
                """PV + normalize + PE transpose per 128-query sub-block."""
                nk = 2 * sq + 2
                work = []
                for h in (0, 1):
                    for qh in (0, 1):
                        # PE stage 1: all four PV chains back-to-back so a
                        # stalled transpose never blocks the next chain
                        # (PE executes in order).
                        pv = ps_sm.tile([P, 65], F32, name="pv", tag="sm")
                        for c in range(nk):
                            nc.tensor.matmul(
                                pv[:],
                                pt[h][:, c, qh * P : (qh + 1) * P],
                                V2[:, b, c, h * 65 : h * 65 + 65],
                                start=(c == 0),
                                stop=(c == nk - 1),
                            )
                        work.append((h, qh, pv))
                osbs = []
                for h, qh, pv in work:
                    # DVE stage: normalize
                    r = rcp.tile([P, 1], F32, name="rr", tag="rr")
                    nc.vector.reciprocal(r[:], pv[:, 0:1])
                    osb = osml.tile([P, DH], BF)
                    nc.vector.tensor_scalar_mul(osb[:], pv[:, 1:65], r[:])
                    osbs.append((h, qh, osb))
                for h, qh, osb in osbs:
                    # PE stage 2 + DVE evict: transpose into projection layout
                    hp = h * DH
                    qb = 2 * sq + qh
                    top = ps_sm.tile([P, P], BF, name="top", tag="sm")
                    nc.tensor.transpose(
                        top[hp : hp + DH, :],
                        osb[:],
                        ident[:],
                        tile_position=(0, hp),
                    )
                    nc.vector.tensor_copy(
                        attn_oT[hp : hp + DH, b * T + qb * P : b * T + (qb + 1) * P],
                        top[hp : hp + DH, :],
                    )

            # Emission: software pipeline with distance 1 — scores(i) are
            # emitted before output-work(i-1) so ACT exps block i while PE
            # chews PV/proj of block i-1; batch-1 qkv rides along batch-0
            # attention.
            for tcg in range(NTC // B):
                qkv_tchunk(tcg)
            blocks = [(0, sq) for sq in range(NSB)] + [(1, sq) for sq in range(NSB)]
            pending = []
            for idx, (b, sq) in enumerate(blocks):
                pt = attn_scores(b, sq)
                pending.append((b, sq, pt))
                if len(pending) > 2:
                    pb, psq, ppt = pending.pop(0)
                    attn_output(pb, psq, ppt)
                    proj_tchunk(pb * (TA // P // B) + 2 * psq)
                    proj_tchunk(pb * (TA // P // B) + 2 * psq + 1)
                if idx < NTC // B:
                    qkv_tchunk(NTC // B + idx)  # batch-1 qkv filler
            for pb, psq, ppt in pending:
                attn_output(pb, psq, ppt)
                proj_tchunk(pb * (TA // P // B) + 2 * psq)
                proj_tchunk(pb * (TA // P // B) + 2 * psq + 1)
            if dbg_out is not None:
                nc.sync.dma_start(dbg_out[:], attn_oT[:])
                nc.sync.dma_start(dbg_pt[:], ppt[1][:].rearrange("p a b -> p (a b)"))

    nc.compile()
    return nc


def get_nc():
    global _CACHED_NC
    if _CACHED_NC is None:
        _CACHED_NC = build_nc()
    return _CACHED_NC


def make_in_maps(x, w_qkv, b_qkv, w_proj):
    x = np.asarray(x, dtype=np.float32).reshape(TA, D)
    w_qkv = np.asarray(w_qkv, dtype=np.float32)
    b_qkv = np.asarray(b_qkv, dtype=np.float32)
    w_proj = np.asarray(w_proj, dtype=np.float32)
    xT = np.ascontiguousarray(x.T)  # [D, TA] fp32, replicated
    # additive causal masks for the two diagonal chunks of a 256-query
    # superblock, in [k_local, q_local] layout: -1600 where the key is
    # ahead of the query (exp(0.125 * -1600) == 0 in fp32).
    kk = np.arange(P)[:, None]
    qq = np.arange(256)[None, :]
    mneg_even = np.where(kk > qq, -1600.0, 0.0)
    mneg_odd = np.where(kk + P > qq, -1600.0, 0.0)
    mask = np.concatenate([mneg_even, mneg_odd], axis=1).astype(ml_dtypes.bfloat16)
    bf = ml_dtypes.bfloat16
    in_maps = []
    for c in range(N_CORES):
        lo = 2 * c * DH  # first feature column of this core's 2 heads
        in_maps.append(
            {
                "xT": xT,
                "wq": np.ascontiguousarray(w_qkv[:, lo : lo + P]).astype(bf),
                "wk": np.ascontiguousarray(w_qkv[:, D + lo : D + lo + P]).astype(bf),
                "wv": np.ascontiguousarray(w_qkv[:, 2 * D + lo : 2 * D + lo + P]).astype(
                    bf
                ),
                "bq": np.ascontiguousarray(b_qkv[lo : lo + P][:, None]),
                "bk": np.ascontiguousarray(b_qkv[D + lo : D + lo + P][:, None]),
                "wp": np.ascontiguousarray(w_proj[lo : lo + P, :]).astype(bf),
                "mask": mask,
            }
        )
    return in_maps


def gather(results, b_qkv, w_proj, b_proj):
    b_qkv = np.asarray(b_qkv, dtype=np.float32)
    w_proj = np.asarray(w_proj, dtype=np.float32)
    b_proj = np.asarray(b_proj, dtype=np.float32)
    y = np.zeros((TA, D), dtype=np.float32)
    for c in range(N_CORES):
        y += np.asarray(results[c]["y"], dtype=np.float32)
    # exact host-side fold of the v-bias and projection bias:
    # softmax rows sum to 1, so the v-bias passes through attention intact.
    y += b_qkv[2 * D : 3 * D] @ w_proj + b_proj
    return y.reshape(B, T, D)


def run(x, w_qkv, b_qkv, w_proj, b_proj, trace=False, **spmd_kwargs):
    nc = get_nc()
    in_maps = make_in_maps(x, w_qkv, b_qkv, w_proj)
    res = run_bass_kernel_spmd(
        nc, in_maps, list(range(N_CORES)), trace=trace, **spmd_kwargs
    )
    return gather(res.results, b_qkv, w_proj, b_proj), res


def kernel(x, w_qkv, b_qkv, w_proj, b_proj):
    y, _ = run(x, w_qkv, b_qkv, w_proj, b_proj)
    return y
